# revision 1
# baseline (speedup 1.0000x reference)
"""GAT-with-edge-features GNN on 8 Trainium2 NeuronCores.

Strategy (self-contained; shapes hardcoded for the fixed problem size):
  - Relabel nodes so each core owns a contiguous block of NPAD node slots,
    grouped graph-wise by batch_vector (core = graph // 8).
  - Partition edges by the owning core of their dst node; within a core,
    sort edges by dst and bin-pack whole dst-segments into 128-edge tiles
    (<= 31 segments per tile) so every segment lives in exactly one tile.
  - Every core redundantly computes the full node-level table B (one matmul
    per 128 nodes) holding [h | mean_h h | h.a_src | h.a_dst] per node, so
    the per-edge gathers h[src]/h[dst] are simple row gathers from HBM
    (gpsimd dma_gather).  Segment softmax is denominators-folded:
    node_out = (sum_e ex*(hs+ep)) / (sum_e ex), accumulated per tile with a
    one-hot segment matmul on the PE, written densely, and gathered back on
    the node side.  Between layers, per-core node states are AllGathered.
"""
import sys
import os

for _p in ("/opt/trn_rl_repo", "/root/.axon_site/_ro/trn_rl_repo"):
    if os.path.isdir(_p) and _p not in sys.path:
        sys.path.insert(0, _p)

import numpy as np

# ---------------- problem constants (hardcoded from spec) ----------------
N_NODES = 10000
N_EDGES = 160000
F_IN = 128
E_IN = 32
F = 64
H = 4
OUT = 64
L_MID = 2
G = 64
NEG_SLOPE = 0.2

# ---------------- sharding constants ----------------
C = 8                 # cores
NPAD = 1408           # padded nodes per core (= 11 * 128)
NT_OWN = NPAD // 128  # 11 node tiles per core
NTOT = C * NPAD       # 11264 global padded node slots
NT_ALL = NTOT // 128  # 88 node tiles total
ET = 192              # edge tiles per core
ES = ET * 128         # 24576 edge slots per core
CH = 8                # edge tiles per gather chunk
NCHUNK = ET // CH     # 24
SEG = 32              # segment slots per tile (31 real + 1 trash)
ACC_CONST_MID = ET * SEG        # 6144: const row for zero-degree nodes (mid)
ACC_CONST_OUT = ET * SEG + 1    # 6145: const row (out layer)
ACC_ROWS = ET * SEG + 64        # padded
WB = 384              # B-table row width, layers 0-2
WB3 = 128             # B-table row width, layer 3
WA = 260              # accum row content width, mid (256 msg + 4 ex)
WA3 = 65              # accum row content width, out (64 msg + 1 ex)

_PROGRAM_CACHE = {}


def _wrap16(u):
    """int index vector [n] -> dma_gather idx layout [128, n//16] int16."""
    n = len(u)
    assert n % 16 == 0
    a = np.asarray(u).reshape(n // 16, 16).T
    return np.ascontiguousarray(np.tile(a, (8, 1)).astype(np.int16))


def _make_wn_aug(Wn, a, with_bias):
    """Wn [fin,H,F], a [3,H,F] -> augmented node weight [fin(+1), WB or WB3]."""
    fin, Hh, Ff = Wn.shape
    w = WB if Hh == 4 else WB3
    Wa = np.zeros((fin + (1 if with_bias else 0), w), np.float32)
    if Hh == 4:
        Wa[:fin, 0:256] = Wn.reshape(fin, 256)
        Wa[:fin, 256:320] = Wn.mean(axis=1)
        Wa[:fin, 320:324] = np.einsum("ihf,hf->ih", Wn, a[0])
        Wa[:fin, 324:328] = np.einsum("ihf,hf->ih", Wn, a[1])
    else:
        Wa[:fin, 0:64] = Wn[:, 0, :]
        Wa[:fin, 64] = np.einsum("if,f->i", Wn[:, 0, :], a[0, 0])
        Wa[:fin, 65] = np.einsum("if,f->i", Wn[:, 0, :], a[1, 0])
    if with_bias:
        Wa[fin, :] = -Wa[:fin, :].sum(axis=0)
    return Wa


def _make_we_aug(We, a, with_bias):
    fin, Hh, Ff = We.shape
    w = WB if Hh == 4 else WB3
    Wa = np.zeros((fin + (1 if with_bias else 0), w), np.float32)
    if Hh == 4:
        Wa[:fin, 0:256] = We.reshape(fin, 256)
        Wa[:fin, 256:320] = We.mean(axis=1)
        Wa[:fin, 320:324] = np.einsum("ihf,hf->ih", We, a[2])
    else:
        Wa[:fin, 0:64] = We[:, 0, :]
        Wa[:fin, 64] = np.einsum("if,f->i", We[:, 0, :], a[2, 0])
    if with_bias:
        Wa[fin, :] = -Wa[:fin, :].sum(axis=0)
    return Wa


def _preprocess(inputs):
    """Host-side: relabel nodes, pack edges, build per-core device inputs."""
    nf = np.asarray(inputs["node_features"], np.float32)
    ef = np.asarray(inputs["edge_features"], np.float32)
    ei = np.asarray(inputs["edge_index"], np.int64)
    bv = np.asarray(inputs["batch_vector"], np.int64)

    core_of_node = bv // (G // C)
    order = np.argsort(bv, kind="stable")          # orig node ids, by (core, graph)
    counts_core = np.bincount(core_of_node, minlength=C)
    assert counts_core.max() <= NPAD, counts_core.max()

    new_id = np.empty(N_NODES, np.int64)
    core_node_lists = []
    off = 0
    for c in range(C):
        ids = order[off:off + counts_core[c]]
        core_node_lists.append(ids)
        new_id[ids] = c * NPAD + np.arange(len(ids))
        off += counts_core[c]

    x0T = np.zeros((F_IN, NTOT), np.float32)
    x0T[:, new_id] = nf.T

    src_new = new_id[ei[0]]
    dst_new = new_id[ei[1]]
    core_of_edge = core_of_node[ei[1]]

    cnt_graph = np.bincount(bv, minlength=G).astype(np.float32)

    per_core = []
    for c in range(C):
        eids = np.nonzero(core_of_edge == c)[0]
        eo = eids[np.argsort(dst_new[eids], kind="stable")]
        d_local = dst_new[eo] - c * NPAD
        uniq, start, cnts = np.unique(d_local, return_index=True, return_counts=True)

        tile_id = 0
        edge_fill = 0
        seg_fill = 0
        slot_of_edge = np.empty(len(eo), np.int64)
        seg_of_edge = np.empty(len(eo), np.int64)
        pos_own = np.full(NPAD, -1, np.int64)
        for u, si, cnt in zip(uniq, start, cnts):
            assert cnt <= 128
            if edge_fill + cnt > 128 or seg_fill >= SEG - 1:
                tile_id += 1
                edge_fill = 0
                seg_fill = 0
            slot_of_edge[si:si + cnt] = tile_id * 128 + edge_fill + np.arange(cnt)
            seg_of_edge[si:si + cnt] = seg_fill
            pos_own[u] = tile_id * SEG + seg_fill
            edge_fill += cnt
            seg_fill += 1
        assert tile_id < ET, tile_id

        slot_src = np.zeros(ES, np.int64)
        slot_src[slot_of_edge] = src_new[eo]
        slot_dst = np.zeros(ES, np.int64)
        slot_dst[slot_of_edge] = dst_new[eo]
        segloc_flat = np.full(ES, SEG - 1, np.float32)
        segloc_flat[slot_of_edge] = seg_of_edge

        e0T_c = np.zeros((E_IN, ES), np.float32)
        e0T_c[:, slot_of_edge] = ef[eo].T

        pos_m = pos_own.copy()
        pos_m[pos_m < 0] = ACC_CONST_MID
        pos_o = pos_own.copy()
        pos_o[pos_o < 0] = ACC_CONST_OUT

        # readout one-hot (scaled by 1/count) for the 8 local graphs
        g1h = np.zeros((128, NT_OWN * 8), np.float32)
        ids = core_node_lists[c]
        if len(ids):
            j = np.arange(len(ids))
            gl = bv[ids] % 8
            vals = 1.0 / np.maximum(cnt_graph[bv[ids]], 1.0)
            g1h[j % 128, (j // 128) * 8 + gl] = vals

        per_core.append(dict(
            e0T=e0T_c,
            srcidx=_wrap16(slot_src),
            dstidx=_wrap16(slot_dst),
            segloc=np.ascontiguousarray(segloc_flat.reshape(ET, 128).T),
            accidx_m=_wrap16(pos_m),
            accidx_o=_wrap16(pos_o),
            g1h=g1h,
        ))

    # ---- weights
    Wn0 = np.asarray(inputs["Wn0"], np.float32)
    We0 = np.asarray(inputs["We0"], np.float32)
    a0 = np.asarray(inputs["a0"], np.float32)
    Wnm = np.asarray(inputs["Wn_mid"], np.float32)
    Wem = np.asarray(inputs["We_mid"], np.float32)
    am = np.asarray(inputs["a_mid"], np.float32)
    Wno = np.asarray(inputs["Wn_out"], np.float32)
    Weo = np.asarray(inputs["We_out"], np.float32)
    ao = np.asarray(inputs["a_out"], np.float32)

    shared = dict(
        x0T=x0T,
        e_ones=np.ones((1, ES), np.float32),
        iota32=np.tile(np.arange(SEG, dtype=np.float32), (128, 1)),
        accconst=np.zeros((2, 320), np.float32),
        wn0=_make_wn_aug(Wn0, a0, False),
        we0=_make_we_aug(We0, a0, False),
        wn1=_make_wn_aug(Wnm[0], am[0], True),
        we1=_make_we_aug(Wem[0], am[0], True),
        wn2=_make_wn_aug(Wnm[1], am[1], True),
        we2=_make_we_aug(Wem[1], am[1], True),
        wn3=_make_wn_aug(Wno, ao, True),
        we3=_make_we_aug(Weo, ao, True),
    )
    shared["accconst"][0, 256:260] = 1.0   # mid const row: num=0, den=1
    shared["accconst"][1, 64] = 1.0        # out const row

    in_maps = []
    for c in range(C):
        m = dict(shared)
        m.update(per_core[c])
        in_maps.append({k: np.ascontiguousarray(v) for k, v in m.items()})
    return in_maps


def _build_program():
    from concourse import bacc, mybir, tile
    from concourse.masks import make_identity

    f32 = mybir.dt.float32
    i16 = mybir.dt.int16
    AOP = mybir.AluOpType
    AF = mybir.ActivationFunctionType

    nc = bacc.Bacc("TRN2", target_bir_lowering=False, debug=False, num_devices=C)

    din = {}
    def dt(name, shape, dtype=f32, kind="ExternalInput"):
        din[name] = nc.dram_tensor(name, shape, dtype, kind=kind)
        return din[name]

    dt("x0T", [F_IN, NTOT])
    dt("e0T", [E_IN, ES])
    dt("e_ones", [1, ES])
    dt("iota32", [128, SEG])
    dt("accconst", [2, 320])
    dt("srcidx", [128, ES // 16], i16)
    dt("dstidx", [128, ES // 16], i16)
    dt("segloc", [128, ET])
    dt("accidx_m", [128, NPAD // 16], i16)
    dt("accidx_o", [128, NPAD // 16], i16)
    dt("g1h", [128, NT_OWN * 8])
    dt("wn0", [F_IN, WB])
    dt("we0", [E_IN, WB])
    for k in (1, 2):
        dt(f"wn{k}", [F + 1, WB])
        dt(f"we{k}", [F + 1, WB])
    dt("wn3", [F + 1, WB3])
    dt("we3", [F + 1, WB3])
    out_t = dt("out", [8, OUT], kind="ExternalOutput")

    with tile.TileContext(nc) as tc:
        with tc.tile_pool(name="persist", bufs=1) as pp, \
             tc.tile_pool(name="work", bufs=2) as wp, \
             tc.tile_pool(name="work3", bufs=3) as wp3, \
             tc.tile_pool(name="pmain", bufs=3, space="PSUM") as pmain, \
             tc.tile_pool(name="pseg", bufs=2, space="PSUM") as pseg, \
             tc.tile_pool(name="ptr", bufs=2, space="PSUM") as ptr, \
             tc.tile_pool(name="pg", bufs=1, space="PSUM") as pgp, \
             tc.tile_pool(name="dram", bufs=1, space="DRAM") as dp:

            # ---- persistent SBUF
            def load_persist(name, shape, dtype=f32):
                t = pp.tile(shape, dtype, tag=name)
                nc.sync.dma_start(out=t[:], in_=din[name].ap())
                return t

            srcidx = load_persist("srcidx", [128, ES // 16], i16)
            dstidx = load_persist("dstidx", [128, ES // 16], i16)
            segloc = load_persist("segloc", [128, ET])
            iota32 = load_persist("iota32", [128, SEG])
            accidx_m = load_persist("accidx_m", [128, NPAD // 16], i16)
            accidx_o = load_persist("accidx_o", [128, NPAD // 16], i16)
            g1h = load_persist("g1h", [128, NT_OWN * 8])
            wn = [load_persist("wn0", [F_IN, WB]),
                  load_persist("wn1", [F + 1, WB]),
                  load_persist("wn2", [F + 1, WB]),
                  load_persist("wn3", [F + 1, WB3])]
            we = [load_persist("we0", [E_IN, WB]),
                  load_persist("we1", [F + 1, WB]),
                  load_persist("we2", [F + 1, WB]),
                  load_persist("we3", [F + 1, WB3])]
            ident = pp.tile([128, 128], f32, tag="ident")
            make_identity(nc, ident[:])

            # ---- DRAM scratch
            B = [dp.tile([NTOT, WB], f32, tag="B0", name="B0"),
                 dp.tile([NTOT, WB], f32, tag="B1", name="B1"),
                 dp.tile([NTOT, WB], f32, tag="B2", name="B2"),
                 dp.tile([NTOT, WB3], f32, tag="B3", name="B3")]
            ACCD = dp.tile([ACC_ROWS, 320], f32, tag="ACC")
            EA = dp.tile([F, ES], f32, tag="EA")
            EB = dp.tile([F, ES], f32, tag="EB")
            AGIN = dp.tile([F + 1, NPAD], f32, tag="AGIN")
            CCT = [dp.tile([C, F + 1, NPAD], f32, tag=f"CC{k}", name=f"CC{k}") for k in range(3)]

            # const rows
            nc.sync.dma_start(out=ACCD[ACC_CONST_MID:ACC_CONST_MID + 2, :],
                              in_=din["accconst"].ap())
            nc.sync.dma_start(out=AGIN[F:F + 1, :], in_=din["e_ones"].ap()[:, 0:NPAD])

            # ------------------------------------------------------------------
            def build_B(L):
                """Full-table node matmul: B[L] rows for all NTOT nodes."""
                WBL = WB if L < 3 else WB3
                for blk in range(C):
                    if L == 0:
                        xb = wp.tile([F_IN, NPAD], f32, tag="xblk")
                        nc.sync.dma_start(out=xb[:], in_=din["x0T"].ap()[:, blk * NPAD:(blk + 1) * NPAD])
                        kn = F_IN
                    else:
                        xb = wp.tile([F + 1, NPAD], f32, tag="xblk")
                        nc.sync.dma_start(out=xb[:], in_=CCT[L - 1][blk, :, :])
                        kn = F + 1
                    for j0 in range(0, NT_OWN, 4):
                        nb = min(4, NT_OWN - j0)
                        bs = wp.tile([128, 4, WB], f32, tag="bstage")
                        for dj in range(nb):
                            j = j0 + dj
                            pb = pmain.tile([128, WB], f32, space="PSUM", tag="pmain")
                            nc.tensor.matmul(out=pb[:, 0:WBL],
                                             lhsT=xb[:kn, j * 128:(j + 1) * 128],
                                             rhs=wn[L][:kn, 0:WBL],
                                             start=True, stop=True)
                            eng = nc.vector if dj % 2 == 0 else nc.scalar
                            if eng is nc.vector:
                                eng.tensor_copy(out=bs[:, dj, 0:WBL], in_=pb[:, 0:WBL])
                            else:
                                eng.copy(out=bs[:, dj, 0:WBL], in_=pb[:, 0:WBL])
                        nt = blk * NT_OWN + j0
                        WR = 328 if L < 3 else WB3
                        nc.sync.dma_start(
                            out=B[L][nt * 128:(nt + nb) * 128, 0:WR].rearrange(
                                "(b p) w -> p b w", p=128),
                            in_=bs[:, 0:nb, 0:WR])

            # ------------------------------------------------------------------
            def edge_phase(L):
                mid = L < 3
                WBL = WB if mid else WB3
                Hk = H if mid else 1
                MS = 256 if mid else 64       # msg cols
                WAk = WA if mid else WA3      # acc row content
                SS = 320 if mid else 64       # psum col of s_src + s_e
                SD = 68 if mid else 1         # GS col of s_dst
                dst_e = 128 if mid else 64    # dst gather elem
                exscale = float(Hk)
                esrc = [None, EA, EB, EA][L]
                edst = [EA, EB, EA, None][L]

                for c in range(NCHUNK):
                    cs, ce = c * CH * 128, (c + 1) * CH * 128
                    gb = wp.tile([128, CH, WBL], f32, tag="gb", name="gb")
                    nc.gpsimd.dma_gather(
                        out_ap=gb[:], in_ap=B[L][:, :],
                        idxs_ap=srcidx[:, c * (CH * 8):(c + 1) * (CH * 8)],
                        num_idxs=CH * 128, num_idxs_reg=CH * 128, elem_size=WBL)
                    gs = wp.tile([128, CH, dst_e], f32, tag="gs", name="gs", bufs=3)
                    nc.gpsimd.dma_gather(
                        out_ap=gs[:],
                        in_ap=B[L][:, WBL - 128:WBL] if mid else B[L][:, 64:128],
                        idxs_ap=dstidx[:, c * (CH * 8):(c + 1) * (CH * 8)],
                        num_idxs=CH * 128, num_idxs_reg=CH * 128,
                        elem_size=dst_e, elem_step=WBL)
                    if L == 0:
                        ech = wp.tile([E_IN, CH * 128], f32, tag="ech0")
                        nc.sync.dma_start(out=ech[:], in_=din["e0T"].ap()[:, cs:ce])
                        ke = E_IN
                    else:
                        ech = wp.tile([F + 1, CH * 128], f32, tag="ech")
                        nc.sync.dma_start(out=ech[0:F, :], in_=esrc[0:F, cs:ce])
                        nc.sync.dma_start(out=ech[F:F + 1, :], in_=din["e_ones"].ap()[:, cs:ce])
                        ke = F + 1
                    acst = wp.tile([SEG, CH, WA], f32, tag="acst")
                    if mid:
                        est = wp.tile([F, CH, 128], f32, tag="est")

                    tch = wp.tile([128, CH, WBL], f32, tag="tch")
                    sohs = []
                    for tl in range(CH):
                        t = c * CH + tl
                        pm = pmain.tile([128, WB], f32, space="PSUM", tag="pmain")
                        nc.tensor.matmul(out=pm[:, 0:WBL],
                                         lhsT=ech[:ke, tl * 128:(tl + 1) * 128],
                                         rhs=we[L][:ke, 0:WBL], start=True, stop=True)
                        # hs + ep (full augmented row), psum released right after
                        nc.vector.tensor_tensor(out=tch[:, tl, :], in0=pm[:, 0:WBL],
                                                in1=gb[:, tl, :], op=AOP.add)
                        # one-hot segment matrix
                        soh = wp3.tile([128, SEG], f32, tag=f"soh{tl}", name="soh")
                        nc.vector.tensor_tensor(
                            out=soh[:], in0=segloc[:, t:t + 1].to_broadcast([128, SEG]),
                            in1=iota32[:], op=AOP.is_equal)
                        sohs.append(soh)
                    # ---- batched small ops over the whole chunk
                    lg = wp3.tile([128, CH, H], f32, tag="lg")
                    nc.vector.tensor_tensor(out=lg[:, :, 0:Hk], in0=tch[:, :, SS:SS + Hk],
                                            in1=gs[:, :, SD:SD + Hk], op=AOP.add)
                    lk = wp3.tile([128, CH, H], f32, tag="lk")
                    nc.vector.tensor_scalar_mul(lk[:, :, 0:Hk], lg[:, :, 0:Hk], NEG_SLOPE)
                    nc.vector.tensor_tensor(out=lk[:, :, 0:Hk], in0=lg[:, :, 0:Hk],
                                            in1=lk[:, :, 0:Hk], op=AOP.max)
                    ex = wp3.tile([128, CH, H], f32, tag="ex")
                    nc.scalar.activation(ex[:, :, 0:Hk], lk[:, :, 0:Hk], AF.Exp)
                    mwch = wp.tile([128, CH, WA], f32, tag="mwch")
                    nc.vector.tensor_tensor(
                        out=mwch[:, :, 0:MS].rearrange("p c (h f) -> p c h f", h=Hk),
                        in0=tch[:, :, 0:MS].rearrange("p c (h f) -> p c h f", h=Hk),
                        in1=ex[:, :, 0:Hk].unsqueeze(3).to_broadcast([128, CH, Hk, F]),
                        op=AOP.mult)
                    nc.vector.tensor_scalar_mul(mwch[:, :, MS:MS + Hk], ex[:, :, 0:Hk], exscale)
                    if mid:
                        eoch = wp.tile([128, CH, F], f32, tag="eoch")
                        nc.vector.tensor_tensor(out=eoch[:], in0=tch[:, :, 256:320],
                                                in1=gs[:, :, 0:F], op=AOP.add)
                        eech = wp.tile([128, CH, F], f32, tag="eech")
                        nc.scalar.activation(eech[:], eoch[:], AF.Exp)
                        rrch = wp.tile([128, CH, F], f32, tag="rrch")
                        nc.vector.tensor_scalar(out=rrch[:], in0=eoch[:], scalar1=0.0,
                                                scalar2=1.0, op0=AOP.max, op1=AOP.add)
                        ench = eech
                        nc.vector.tensor_tensor(out=ench[:], in0=eech[:], in1=rrch[:], op=AOP.min)
                    for tl in range(CH):
                        t = c * CH + tl
                        ps = pseg.tile([SEG, WA], f32, space="PSUM", tag="pseg")
                        nc.tensor.matmul(out=ps[:, 0:WAk], lhsT=sohs[tl][:],
                                         rhs=mwch[:, tl, 0:WAk], start=True, stop=True)
                        if tl % 2 == 0:
                            nc.scalar.copy(out=acst[:, tl, 0:WAk], in_=ps[:, 0:WAk])
                        else:
                            nc.vector.tensor_copy(out=acst[:, tl, 0:WAk], in_=ps[:, 0:WAk])
                        if mid:
                            pt = ptr.tile([F, 128], f32, space="PSUM", tag="ptr")
                            nc.tensor.transpose(out=pt[:], in_=ench[:, tl, :], identity=ident[:])
                            if tl % 2 == 0:
                                nc.vector.tensor_copy(out=est[:, tl, :], in_=pt[:])
                            else:
                                nc.scalar.copy(out=est[:, tl, :], in_=pt[:])
                    nc.sync.dma_start(
                        out=ACCD[c * CH * SEG:(c + 1) * CH * SEG, 0:WAk].rearrange(
                            "(t s) w -> s t w", s=SEG),
                        in_=acst[:, :, 0:WAk])
                    if mid:
                        nc.sync.dma_start(
                            out=edst[0:F, cs:ce].rearrange("f (t p) -> f t p", p=128),
                            in_=est[:])

            # ------------------------------------------------------------------
            def node_phase(L, sub="c"):
                if L < 3:
                    gn = wp.tile([128, NT_OWN, 320], f32, tag="gn", bufs=1)
                    nc.gpsimd.dma_gather(
                        out_ap=gn[:, 0:8, :], in_ap=ACCD[:, :], idxs_ap=accidx_m[:, 0:64],
                        num_idxs=1024, num_idxs_reg=1024, elem_size=320)
                    nc.gpsimd.dma_gather(
                        out_ap=gn[:, 8:NT_OWN, :], in_ap=ACCD[:, :], idxs_ap=accidx_m[:, 64:88],
                        num_idxs=NPAD - 1024, num_idxs_reg=NPAD - 1024, elem_size=320)
                    xstg = wp.tile([F, NT_OWN, 128], f32, tag="xstg", bufs=1)
                    for nt in range(NT_OWN):
                        rec = wp3.tile([128, H], f32, tag="rec")
                        nc.vector.reciprocal(out=rec[:], in_=gn[:, nt, 256:260])
                        pr = wp3.tile([128, 256], f32, tag="pr")
                        nc.vector.tensor_tensor(
                            out=pr[:].rearrange("p (h f) -> p h f", h=H),
                            in0=gn[:, nt, 0:256].rearrange("p (h f) -> p h f", h=H),
                            in1=rec[:].unsqueeze(2).to_broadcast([128, H, F]),
                            op=AOP.mult)
                        xo = wp3.tile([128, F], f32, tag="xo")
                        nc.vector.tensor_reduce(
                            out=xo[:], in_=pr[:].rearrange("p (h f) -> p f h", h=H),
                            axis=mybir.AxisListType.X, op=AOP.add)
                        xe = wp3.tile([128, F], f32, tag="ee")
                        nc.scalar.activation(xe[:], xo[:], AF.Exp)
                        xr = wp3.tile([128, F], f32, tag="rr")
                        nc.vector.tensor_scalar(out=xr[:], in0=xo[:], scalar1=0.0,
                                                scalar2=1.0, op0=AOP.max, op1=AOP.add)
                        xs = wp3.tile([128, F], f32, tag="en")
                        nc.vector.tensor_tensor(out=xs[:], in0=xe[:], in1=xr[:], op=AOP.min)
                        pt = ptr.tile([F, 128], f32, space="PSUM", tag="ptr")
                        nc.tensor.transpose(out=pt[:], in_=xs[:], identity=ident[:])
                        if nt % 2 == 0:
                            nc.vector.tensor_copy(out=xstg[:, nt, :], in_=pt[:])
                        else:
                            nc.scalar.copy(out=xstg[:, nt, :], in_=pt[:])
                    nc.sync.dma_start(
                        out=AGIN[0:F, :].rearrange("f (t p) -> f t p", p=128),
                        in_=xstg[:])
                    if sub == "a":
                        return
                    nc.gpsimd.collective_compute(
                        "AllGather", AOP.bypass,
                        replica_groups=[list(range(C))],
                        ins=[AGIN[:]], outs=[CCT[L][:]])
                    if sub == "b":
                        return
                    build_B(L + 1)
                else:
                    gn = wp.tile([128, NT_OWN, 128], f32, tag="gn", name="gn3", bufs=1)
                    nc.gpsimd.dma_gather(
                        out_ap=gn[:, 0:8, :], in_ap=ACCD[:, 0:128], idxs_ap=accidx_o[:, 0:64],
                        num_idxs=1024, num_idxs_reg=1024, elem_size=128, elem_step=320)
                    nc.gpsimd.dma_gather(
                        out_ap=gn[:, 8:NT_OWN, :], in_ap=ACCD[:, 0:128], idxs_ap=accidx_o[:, 64:88],
                        num_idxs=NPAD - 1024, num_idxs_reg=NPAD - 1024, elem_size=128, elem_step=320)
                    pg = pgp.tile([8, OUT], f32, space="PSUM", tag="pg")
                    for nt in range(NT_OWN):
                        rec = wp3.tile([128, H], f32, tag="rec")
                        nc.vector.reciprocal(out=rec[:, 0:1], in_=gn[:, nt, 64:65])
                        nod = wp3.tile([128, 256], f32, tag="pr")
                        nc.vector.tensor_tensor(out=nod[:, 0:OUT], in0=gn[:, nt, 0:OUT],
                                                in1=rec[:, 0:1].to_broadcast([128, OUT]),
                                                op=AOP.mult)
                        nc.tensor.matmul(out=pg[:], lhsT=g1h[:, nt * 8:(nt + 1) * 8],
                                         rhs=nod[:, 0:OUT], start=(nt == 0),
                                         stop=(nt == NT_OWN - 1), skip_group_check=True)
                    og = wp3.tile([8, OUT], f32, tag="og")
                    nc.vector.tensor_copy(out=og[:], in_=pg[:])
                    nc.sync.dma_start(out=out_t.ap(), in_=og[:])

            # ------------------------------------------------------------------
            stage = os.environ.get("KERNEL_STAGE", "full")
            og0 = wp3.tile([8, OUT], f32, tag="og", name="og0")
            nc.vector.memset(og0[:], 0.0)
            nc.sync.dma_start(out=out_t.ap(), in_=og0[:])
            if stage == "full":
                build_B(0)
                for L in range(4):
                    edge_phase(L)
                    node_phase(L)
            elif stage in ("3a", "3b"):
                build_B(0)
                edge_phase(0)
                node_phase(0, sub=stage[-1])
            elif stage == "3g":
                build_B(0)
                edge_phase(0)
                gn = wp.tile([128, NT_OWN, 320], f32, tag="gn", name="gng")
                nc.gpsimd.dma_gather(
                    out_ap=gn[:], in_ap=ACCD[:, :], idxs_ap=accidx_m[:],
                    num_idxs=NPAD, num_idxs_reg=NPAD, elem_size=320)
                nc.vector.tensor_copy(out=og0[:], in_=gn[0:8, 0, 0:OUT])
                nc.sync.dma_start(out=out_t.ap(), in_=og0[:])
            elif stage == "3m":
                build_B(0)
                edge_phase(0)
                node_phase_math_only = True
                gn = wp.tile([128, NT_OWN, 320], f32, tag="gn", name="gnm")
                nc.gpsimd.dma_gather(
                    out_ap=gn[:], in_ap=ACCD[:, :], idxs_ap=accidx_m[:],
                    num_idxs=NPAD, num_idxs_reg=NPAD, elem_size=320)
                for nt in range(NT_OWN):
                    rec = wp3.tile([128, H], f32, tag="rec", name="recm")
                    nc.vector.reciprocal(out=rec[:], in_=gn[:, nt, 256:260])
                    pr = wp3.tile([128, 256], f32, tag="pr", name="prm")
                    nc.vector.tensor_tensor(
                        out=pr[:].rearrange("p (h f) -> p h f", h=H),
                        in0=gn[:, nt, 0:256].rearrange("p (h f) -> p h f", h=H),
                        in1=rec[:].unsqueeze(2).to_broadcast([128, H, F]),
                        op=AOP.mult)
                    xo = wp3.tile([128, F], f32, tag="xo", name="xom")
                    nc.vector.tensor_reduce(
                        out=xo[:], in_=pr[:].rearrange("p (h f) -> p f h", h=H),
                        axis=mybir.AxisListType.X, op=AOP.add)
                nc.vector.tensor_copy(out=og0[:], in_=gn[0:8, 0, 0:OUT])
                nc.sync.dma_start(out=out_t.ap(), in_=og0[:])
            else:
                n = int(stage)  # 1=B0, 2=+edge0, 3=+node0, 4=+edge1, 5=+node1, ...
                step = 0
                done = False
                build_B(0)
                step += 1
                for L in range(4):
                    if step >= n:
                        done = True
                        break
                    edge_phase(L)
                    step += 1
                    if step >= n:
                        done = True
                        break
                    node_phase(L)
                    step += 1

    nc.compile()
    return nc


def _get_program():
    if "nc" not in _PROGRAM_CACHE:
        _PROGRAM_CACHE["nc"] = _build_program()
    return _PROGRAM_CACHE["nc"]


def kernel(**inputs):
    from concourse.bass_utils import run_bass_kernel_spmd

    nc = _get_program()
    in_maps = _preprocess(inputs)
    trace = bool(int(os.environ.get("KERNEL_TRACE", "0")))
    res = run_bass_kernel_spmd(nc, in_maps, core_ids=list(range(C)), trace=trace)
    _PROGRAM_CACHE["last_result"] = res
    out = np.concatenate([np.asarray(res.results[c]["out"]) for c in range(C)], axis=0)
    return out.astype(np.float32)



# revision 7
# speedup vs baseline: 1.8202x; 1.8202x over previous
"""GAT-with-edge-features GNN on 8 Trainium2 NeuronCores.

Strategy (self-contained; shapes hardcoded for the fixed problem size):
  - Relabel nodes so each core owns a contiguous block of NPAD node slots,
    grouped graph-wise by batch_vector (core = graph // 8).
  - Partition edges by the owning core of their dst node; within a core,
    sort edges by dst and FFD-bin-pack whole dst-segments into 128-edge
    tiles (<= 31 segments per tile) so every segment lives in one tile.
  - Every core redundantly computes the full node-level table B (fp16) so
    per-edge h[src] is a row gather from HBM (gpsimd dma_gather).  The
    dst-side values (mean_h_d, s_dst) are NOT gathered per edge: they are
    kept in a slot-ordered table DSTTAB[L] (one row per (tile,seg)) built
    by a 1408-row dma_scatter_add during the previous node phase, then
    broadcast to edges with a per-tile one-hot matmul fused into the same
    PSUM as the edge-feature projection.  Segment softmax is
    denominators-folded: node_out = (sum_e ex*(hs+ep)) / (sum_e ex),
    accumulated per tile with a one-hot segment matmul on the PE.
    Between layers, per-core node states are AllGathered (fp16, Shared).
"""
import sys
import os

for _p in ("/opt/trn_rl_repo", "/root/.axon_site/_ro/trn_rl_repo"):
    if os.path.isdir(_p) and _p not in sys.path:
        sys.path.insert(0, _p)

import numpy as np

# ---------------- problem constants (hardcoded from spec) ----------------
N_NODES = 10000
N_EDGES = 160000
F_IN = 128
E_IN = 32
F = 64
H = 4
OUT = 64
L_MID = 2
G = 64
NEG_SLOPE = 0.2

# ---------------- sharding constants ----------------
C = 8                 # cores
NPAD = 1408           # padded nodes per core (= 11 * 128)
NT_OWN = NPAD // 128  # 11 node tiles per core
NTOT = C * NPAD       # 11264 global padded node slots
ET = 168              # edge tiles per core (FFD-packed; assert fits)
ES = ET * 128         # 21504 edge slots per core
CH = 8                # edge tiles per gather chunk
NCHUNK = ET // CH     # 21
SEG = 32              # segment slots per tile (31 real + 1 trash)
ACCN = ET * SEG       # 5376 accumulation rows
ACC_CONST_MID = ACCN          # const row for zero-degree nodes (mid)
ACC_CONST_OUT = ACCN + 1      # const row (out layer)
ACC_ROWS = ACCN + 64
WB = 384              # B-table row width, layers 0-2
WB3 = 128             # B-table row width, layer 3
WA = 260              # accum row content width, mid (256 msg + 4 ex)
WA3 = 65              # accum row content width, out (64 msg + 1 ex)

_PROGRAM_CACHE = {}


def _wrap16(u):
    """int index vector [n] -> dma_gather idx layout [128, n//16] int16."""
    n = len(u)
    assert n % 16 == 0
    a = np.asarray(u).reshape(n // 16, 16).T
    return np.ascontiguousarray(np.tile(a, (8, 1)).astype(np.int16))


def _make_wn_aug(Wn, a):
    """Wn [fin,H,F], a [3,H,F] -> augmented node weight [fin, WB or WB3]."""
    fin, Hh, Ff = Wn.shape
    w = WB if Hh == 4 else WB3
    Wa = np.zeros((fin, w), np.float32)
    if Hh == 4:
        Wa[:, 0:256] = Wn.reshape(fin, 256)
        Wa[:, 256:320] = Wn.mean(axis=1)
        Wa[:, 320:324] = np.einsum("ihf,hf->ih", Wn, a[0])
    else:
        Wa[:, 0:64] = Wn[:, 0, :]
        Wa[:, 64] = np.einsum("if,f->i", Wn[:, 0, :], a[0, 0])
    return Wa


def _make_we_aug(We, a):
    fin, Hh, Ff = We.shape
    w = WB if Hh == 4 else WB3
    Wa = np.zeros((fin, w), np.float32)
    if Hh == 4:
        Wa[:, 0:256] = We.reshape(fin, 256)
        Wa[:, 256:320] = We.mean(axis=1)
        Wa[:, 320:324] = np.einsum("ihf,hf->ih", We, a[2])
    else:
        Wa[:, 0:64] = We[:, 0, :]
        Wa[:, 64] = np.einsum("if,f->i", We[:, 0, :], a[2, 0])
    return Wa


def _make_wdst(Wn, a):
    """dst-side row projector: x_d -> [mean_h_d(0:64) | s_dst(64:64+H) | 0]."""
    fin, Hh, Ff = Wn.shape
    Wa = np.zeros((fin, 128), np.float32)
    if Hh == 4:
        Wa[:, 0:64] = Wn.mean(axis=1)
        Wa[:, 64:68] = np.einsum("ihf,hf->ih", Wn, a[1])
    else:
        # out layer: edge_out unused, only the score is needed
        Wa[:, 64] = np.einsum("if,f->i", Wn[:, 0, :], a[1, 0])
    return Wa


def _f16(x):
    return np.ascontiguousarray(np.asarray(x, np.float32).astype(np.float16))


def _pack_core(dst_local_sorted):
    """FFD-pack whole dst-segments into tiles of <=128 edges, <=31 segments.

    dst_local_sorted: dst node (orig id) per edge, sorted ascending.
    Returns (tile_of_seg, segidx_of_seg, uniq, start, cnts, tile_edges)
    where seg i is the i-th unique dst in ascending-dst order.
    """
    uniq, start, cnts = np.unique(dst_local_sorted, return_index=True,
                                  return_counts=True)
    nseg = len(uniq)
    order = np.argsort(-cnts, kind="stable")   # big segments first
    tile_edges = []
    tile_nseg = []
    tile_of_seg = np.empty(nseg, np.int64)
    segidx_of_seg = np.empty(nseg, np.int64)
    for si in order:
        cnt = cnts[si]
        placed = False
        for t in range(len(tile_edges)):
            if tile_edges[t] + cnt <= 128 and tile_nseg[t] < SEG - 1:
                tile_of_seg[si] = t
                segidx_of_seg[si] = tile_nseg[t]
                tile_edges[t] += cnt
                tile_nseg[t] += 1
                placed = True
                break
        if not placed:
            tile_of_seg[si] = len(tile_edges)
            segidx_of_seg[si] = 0
            tile_edges.append(int(cnt))
            tile_nseg.append(1)
    assert len(tile_edges) <= ET, len(tile_edges)
    return tile_of_seg, segidx_of_seg, uniq, start, cnts, tile_edges


def _preprocess(inputs):
    """Host-side: relabel nodes, pack edges, build per-core device inputs."""
    nf = np.asarray(inputs["node_features"], np.float32)
    ef = np.asarray(inputs["edge_features"], np.float32)
    ei = np.asarray(inputs["edge_index"], np.int64)
    bv = np.asarray(inputs["batch_vector"], np.int64)

    core_of_node = bv // (G // C)
    core_of_edge = core_of_node[ei[1]]
    cnt_graph = np.bincount(bv, minlength=G).astype(np.float32)

    # ---- per-core packing (uses orig ids), then global relabel
    packs = []
    new_id = np.empty(N_NODES, np.int64)
    core_node_lists = []
    for c in range(C):
        eids = np.nonzero(core_of_edge == c)[0]
        eo = eids[np.argsort(ei[1][eids], kind="stable")]
        dsts = ei[1][eo]
        tile_of_seg, segidx_of_seg, uniq, start, cnts, tile_edges = \
            _pack_core(dsts)
        nseg = len(uniq)

        ntile = len(tile_edges)
        seg_order = np.lexsort((segidx_of_seg, tile_of_seg))
        off_in_tile = np.zeros(nseg, np.int64)
        fill = np.zeros(ntile, np.int64)
        for si in seg_order:
            t = tile_of_seg[si]
            off_in_tile[si] = fill[t]
            fill[t] += cnts[si]

        slot_of_edge = np.empty(len(eo), np.int64)
        seg_of_edge = np.empty(len(eo), np.int64)
        for si in range(nseg):
            s0 = start[si]
            cnt = cnts[si]
            slot_of_edge[s0:s0 + cnt] = (tile_of_seg[si] * 128
                                         + off_in_tile[si] + np.arange(cnt))
            seg_of_edge[s0:s0 + cnt] = segidx_of_seg[si]

        # node order: nodes with segments by (tile, segidx), then the rest
        nodes_with_seg = uniq[seg_order]
        own = np.nonzero(core_of_node == c)[0]
        rest = own[~np.isin(own, nodes_with_seg)]
        ordered = np.concatenate([nodes_with_seg, rest])
        assert len(ordered) <= NPAD, len(ordered)
        new_id[ordered] = c * NPAD + np.arange(len(ordered))
        core_node_lists.append(ordered)
        packs.append(dict(eo=eo, slot_of_edge=slot_of_edge,
                          seg_of_edge=seg_of_edge, nseg=nseg,
                          tile_of_seg=tile_of_seg, segidx_of_seg=segidx_of_seg,
                          seg_order=seg_order))

    x0T = np.zeros((F_IN, NTOT), np.float32)
    x0T[:, new_id] = nf.T
    src_new = new_id[ei[0]]

    per_core = []
    for c in range(C):
        p = packs[c]
        eo, slot_of_edge, seg_of_edge = p["eo"], p["slot_of_edge"], p["seg_of_edge"]
        nseg = p["nseg"]

        slot_src = np.zeros(ES, np.int64)
        slot_src[slot_of_edge] = src_new[eo]
        seg_of_slot = np.full(ES, SEG - 1, np.int64)
        seg_of_slot[slot_of_edge] = seg_of_edge

        # one-hot segment matrices (static): soh [128, ET*SEG], sohT [SEG, ET*128]
        soh = np.zeros((128, ET * SEG), np.float32)
        sohT = np.zeros((SEG, ET * 128), np.float32)
        sl = np.arange(ES)
        soh[sl % 128, (sl // 128) * SEG + seg_of_slot] = 1.0
        sohT[seg_of_slot, sl] = 1.0

        # node k (k-th in core order) -> ACC row / DSTTAB slot
        accslot = np.full(NPAD, -1, np.int64)
        so = p["seg_order"]
        accslot[0:nseg] = p["tile_of_seg"][so] * SEG + p["segidx_of_seg"][so]
        pos_m = accslot.copy()
        pos_m[pos_m < 0] = ACC_CONST_MID
        pos_o = accslot.copy()
        pos_o[pos_o < 0] = ACC_CONST_OUT
        dstscat = accslot.copy()
        dstscat[dstscat < 0] = ACCN       # trash row, never read back

        e0T_c = np.zeros((E_IN, ES), np.float32)
        e0T_c[:, slot_of_edge] = ef[eo].T

        # readout one-hot (scaled by 1/count) for the 8 local graphs
        g1h = np.zeros((128, NT_OWN * 8), np.float32)
        ids = core_node_lists[c]
        if len(ids):
            j = np.arange(len(ids))
            gl = bv[ids] % 8
            vals = 1.0 / np.maximum(cnt_graph[bv[ids]], 1.0)
            g1h[j % 128, (j // 128) * 8 + gl] = vals

        per_core.append(dict(
            e0T=_f16(e0T_c),
            srcidx=_wrap16(slot_src),
            soh=_f16(soh),
            sohT=_f16(sohT),
            accidx_m=_wrap16(pos_m),
            accidx_o=_wrap16(pos_o),
            dstscat=_wrap16(dstscat),
            g1h=g1h,
            x0own=_f16(x0T[:, c * NPAD:(c + 1) * NPAD]),
        ))

    # ---- weights
    Wn0 = np.asarray(inputs["Wn0"], np.float32)
    We0 = np.asarray(inputs["We0"], np.float32)
    a0 = np.asarray(inputs["a0"], np.float32)
    Wnm = np.asarray(inputs["Wn_mid"], np.float32)
    Wem = np.asarray(inputs["We_mid"], np.float32)
    am = np.asarray(inputs["a_mid"], np.float32)
    Wno = np.asarray(inputs["Wn_out"], np.float32)
    Weo = np.asarray(inputs["We_out"], np.float32)
    ao = np.asarray(inputs["a_out"], np.float32)

    shared = dict(
        x0T=_f16(x0T),
        accconst=np.zeros((2, 320), np.float32),
        wn0=_f16(_make_wn_aug(Wn0, a0)),
        we0=_f16(_make_we_aug(We0, a0)),
        wn1=_f16(_make_wn_aug(Wnm[0], am[0])),
        we1=_f16(_make_we_aug(Wem[0], am[0])),
        wn2=_f16(_make_wn_aug(Wnm[1], am[1])),
        we2=_f16(_make_we_aug(Wem[1], am[1])),
        wn3=_f16(_make_wn_aug(Wno, ao)),
        we3=_f16(_make_we_aug(Weo, ao)),
        wd0=_f16(_make_wdst(Wn0, a0)),
        wd1=_f16(_make_wdst(Wnm[0], am[0])),
        wd2=_f16(_make_wdst(Wnm[1], am[1])),
        wd3=_f16(_make_wdst(Wno, ao)),
    )
    shared["accconst"][0, 256:260] = 1.0   # mid const row: num=0, den=1
    shared["accconst"][1, 64] = 1.0        # out const row

    in_maps = []
    for c in range(C):
        m = dict(shared)
        m.update(per_core[c])
        in_maps.append({k: np.ascontiguousarray(v) for k, v in m.items()})
    return in_maps


def _build_program():
    from concourse import bacc, mybir, tile
    from concourse.masks import make_identity

    f32 = mybir.dt.float32
    f16 = mybir.dt.float16
    i16 = mybir.dt.int16
    AOP = mybir.AluOpType
    AF = mybir.ActivationFunctionType

    nc = bacc.Bacc("TRN2", target_bir_lowering=False, debug=False, num_devices=C)

    din = {}
    def dt(name, shape, dtype=f32, kind="ExternalInput"):
        din[name] = nc.dram_tensor(name, shape, dtype, kind=kind)
        return din[name]

    dt("x0T", [F_IN, NTOT], f16)
    dt("x0own", [F_IN, NPAD], f16)
    dt("e0T", [E_IN, ES], f16)
    dt("accconst", [2, 320])
    dt("srcidx", [128, ES // 16], i16)
    dt("soh", [128, ET * SEG], f16)
    dt("sohT", [SEG, ET * 128], f16)
    dt("accidx_m", [128, NPAD // 16], i16)
    dt("accidx_o", [128, NPAD // 16], i16)
    dt("dstscat", [128, NPAD // 16], i16)
    dt("g1h", [128, NT_OWN * 8])
    dt("wn0", [F_IN, WB], f16)
    dt("we0", [E_IN, WB], f16)
    for k in (1, 2):
        dt(f"wn{k}", [F, WB], f16)
        dt(f"we{k}", [F, WB], f16)
    dt("wn3", [F, WB3], f16)
    dt("we3", [F, WB3], f16)
    dt("wd0", [F_IN, 128], f16)
    for k in (1, 2, 3):
        dt(f"wd{k}", [F, 128], f16)
    out_t = dt("out", [8, OUT], kind="ExternalOutput")

    with tile.TileContext(nc) as tc:
        with tc.tile_pool(name="persist", bufs=1) as pp, \
             tc.tile_pool(name="work", bufs=2) as wp, \
             tc.tile_pool(name="work3", bufs=3) as wp3, \
             tc.tile_pool(name="pmain", bufs=3, space="PSUM") as pmain, \
             tc.tile_pool(name="pseg", bufs=2, space="PSUM") as pseg, \
             tc.tile_pool(name="ptr", bufs=2, space="PSUM") as ptr, \
             tc.tile_pool(name="pg", bufs=1, space="PSUM") as pgp, \
             tc.tile_pool(name="dram", bufs=1, space="DRAM") as dp:

            # ---- persistent SBUF
            def load_persist(name, shape, dtype=f32):
                t = pp.tile(shape, dtype, tag=name)
                nc.sync.dma_start(out=t[:], in_=din[name].ap())
                return t

            srcidx = load_persist("srcidx", [128, ES // 16], i16)
            soh = load_persist("soh", [128, ET * SEG], f16)
            sohT = load_persist("sohT", [SEG, ET * 128], f16)
            accidx_m = load_persist("accidx_m", [128, NPAD // 16], i16)
            accidx_o = load_persist("accidx_o", [128, NPAD // 16], i16)
            dstscat = load_persist("dstscat", [128, NPAD // 16], i16)
            g1h = load_persist("g1h", [128, NT_OWN * 8])
            x0own = load_persist("x0own", [F_IN, NPAD], f16)
            wn = [load_persist("wn0", [F_IN, WB], f16),
                  load_persist("wn1", [F, WB], f16),
                  load_persist("wn2", [F, WB], f16),
                  load_persist("wn3", [F, WB3], f16)]
            we = [load_persist("we0", [E_IN, WB], f16),
                  load_persist("we1", [F, WB], f16),
                  load_persist("we2", [F, WB], f16),
                  load_persist("we3", [F, WB3], f16)]
            wd = [load_persist("wd0", [F_IN, 128], f16),
                  load_persist("wd1", [F, 128], f16),
                  load_persist("wd2", [F, 128], f16),
                  load_persist("wd3", [F, 128], f16)]
            ident = pp.tile([128, 128], f16, tag="ident")
            make_identity(nc, ident[:])

            # ---- DRAM scratch
            B = [dp.tile([NTOT, WB], f16, tag="B0", name="B0"),
                 dp.tile([NTOT, WB], f16, tag="B1", name="B1"),
                 dp.tile([NTOT, WB], f16, tag="B2", name="B2"),
                 dp.tile([NTOT, WB3], f16, tag="B3", name="B3")]
            ACCD = dp.tile([ACC_ROWS, 320], f32, tag="ACC")
            EA = dp.tile([F, ES], f16, tag="EA")
            EB = dp.tile([F, ES], f16, tag="EB")
            AGIN = dp.tile([F, NPAD], f16, tag="AGIN")
            CCT = [dp.tile([C, F, NPAD], f16, tag=f"CC{k}", name=f"CC{k}",
                           addr_space="Shared") for k in range(3)]
            DTB = [dp.tile([ACCN + 64, 128], f16, tag=f"DT{k}", name=f"DT{k}")
                   for k in range(4)]

            # const rows + zero-init of DSTTABs
            nc.sync.dma_start(out=ACCD[ACC_CONST_MID:ACC_CONST_MID + 2, :],
                              in_=din["accconst"].ap())
            zeros = pp.tile([128, ACCN], f16, tag="zeros")
            nc.vector.memset(zeros[:], 0.0)
            for k in range(4):
                nc.sync.dma_start(
                    out=DTB[k][0:ACCN, :].rearrange("(a p) w -> p a w", p=128),
                    in_=zeros[:].rearrange("p (a w) -> p a w", w=128))

            # ------------------------------------------------------------------
            def build_dst0():
                """DSTTAB[0] rows from raw node features (own block)."""
                dstg = wp.tile([128, NT_OWN, 128], f16, tag="dstg", bufs=1,
                               name="dstg0")
                for nt in range(NT_OWN):
                    pd = pmain.tile([128, WB], f32, space="PSUM", tag="pmain")
                    nc.tensor.matmul(out=pd[:, 0:128],
                                     lhsT=x0own[:, nt * 128:(nt + 1) * 128],
                                     rhs=wd[0][:, :], start=True, stop=True)
                    if nt % 2 == 0:
                        nc.vector.tensor_copy(out=dstg[:, nt, :], in_=pd[:, 0:128])
                    else:
                        nc.scalar.copy(out=dstg[:, nt, :], in_=pd[:, 0:128])
                nc.gpsimd.dma_scatter_add(
                    out_ap=DTB[0][:, :], in_ap=dstg[:], idxs_ap=dstscat[:],
                    num_idxs=NPAD, num_idxs_reg=NPAD, elem_size=128)

            # ------------------------------------------------------------------
            def build_B(L):
                """Full-table node matmul: B[L] rows for all NTOT nodes."""
                WBL = WB if L < 3 else WB3
                for blk in range(C):
                    if L == 0:
                        xb = wp.tile([F_IN, NPAD], f16, tag="xblk")
                        nc.sync.dma_start(out=xb[:], in_=din["x0T"].ap()[:, blk * NPAD:(blk + 1) * NPAD])
                        kn = F_IN
                    else:
                        xb = wp.tile([F, NPAD], f16, tag="xblk", name="xblkm")
                        nc.sync.dma_start(out=xb[:], in_=CCT[L - 1][blk, :, :])
                        kn = F
                    for j0 in range(0, NT_OWN, 4):
                        nb = min(4, NT_OWN - j0)
                        bs = wp.tile([128, 4, WB], f16, tag="bstage")
                        for dj in range(nb):
                            j = j0 + dj
                            pb = pmain.tile([128, WB], f32, space="PSUM", tag="pmain")
                            nc.tensor.matmul(out=pb[:, 0:WBL],
                                             lhsT=xb[:kn, j * 128:(j + 1) * 128],
                                             rhs=wn[L][:kn, 0:WBL],
                                             start=True, stop=True)
                            if dj % 2 == 0:
                                nc.vector.tensor_copy(out=bs[:, dj, 0:WBL], in_=pb[:, 0:WBL])
                            else:
                                nc.scalar.copy(out=bs[:, dj, 0:WBL], in_=pb[:, 0:WBL])
                        nt = blk * NT_OWN + j0
                        nc.sync.dma_start(
                            out=B[L][nt * 128:(nt + nb) * 128, 0:WBL].rearrange(
                                "(b p) w -> p b w", p=128),
                            in_=bs[:, 0:nb, 0:WBL])

            # ------------------------------------------------------------------
            def edge_phase(L):
                mid = L < 3
                WBL = WB if mid else WB3
                Hk = H if mid else 1
                MS = 256 if mid else 64       # msg cols
                WAk = WA if mid else WA3      # acc row content
                SS = 320 if mid else 64       # col of summed scores
                DOFF = 256 if mid else 0      # psum col where dst-mm lands
                exscale = float(Hk)
                esrc = [None, EA, EB, EA][L]
                edst = [EA, EB, EA, None][L]

                for c in range(NCHUNK):
                    cs, ce = c * CH * 128, (c + 1) * CH * 128
                    gb = wp.tile([128, CH, WBL], f16, tag="gb", name="gb")
                    nc.gpsimd.dma_gather(
                        out_ap=gb[:], in_ap=B[L][:, :],
                        idxs_ap=srcidx[:, c * (CH * 8):(c + 1) * (CH * 8)],
                        num_idxs=CH * 128, num_idxs_reg=CH * 128, elem_size=WBL)
                    dr = wp.tile([SEG, CH, 128], f16, tag="dr", name="dr")
                    nc.sync.dma_start(
                        out=dr[:],
                        in_=DTB[L][c * CH * SEG:(c + 1) * CH * SEG, :].rearrange(
                            "(t s) w -> s t w", s=SEG))
                    if L == 0:
                        ech = wp.tile([E_IN, CH * 128], f16, tag="ech0")
                        nc.sync.dma_start(out=ech[:], in_=din["e0T"].ap()[:, cs:ce])
                        ke = E_IN
                    else:
                        ech = wp.tile([F, CH * 128], f16, tag="ech")
                        nc.sync.dma_start(out=ech[:], in_=esrc[0:F, cs:ce])
                        ke = F
                    acst = wp.tile([SEG, CH, WA], f32, tag="acst")
                    if mid:
                        est = wp.tile([F, CH, 128], f16, tag="est")

                    tch = wp.tile([128, CH, WBL], f16, tag="tch")
                    for tl in range(CH):
                        t = c * CH + tl
                        pm = pmain.tile([128, WB], f32, space="PSUM", tag="pmain")
                        nc.tensor.matmul(out=pm[:, 0:WBL],
                                         lhsT=ech[:ke, tl * 128:(tl + 1) * 128],
                                         rhs=we[L][:ke, 0:WBL], start=True,
                                         stop=False, skip_group_check=True)
                        nc.tensor.matmul(out=pm[:, DOFF:DOFF + 128],
                                         lhsT=sohT[:, t * 128:(t + 1) * 128],
                                         rhs=dr[:, tl, :], start=False,
                                         stop=True, skip_group_check=True)
                        # hs + (ep + dst-bcast), psum released right after
                        nc.vector.tensor_tensor(out=tch[:, tl, :], in0=pm[:, 0:WBL],
                                                in1=gb[:, tl, :], op=AOP.add)
                    # ---- batched small ops over the whole chunk
                    lk = wp3.tile([128, CH, H], f16, tag="lk")
                    nc.vector.tensor_scalar_mul(lk[:, :, 0:Hk], tch[:, :, SS:SS + Hk],
                                                NEG_SLOPE)
                    nc.vector.tensor_tensor(out=lk[:, :, 0:Hk], in0=tch[:, :, SS:SS + Hk],
                                            in1=lk[:, :, 0:Hk], op=AOP.max)
                    ex = wp3.tile([128, CH, H], f16, tag="ex")
                    nc.scalar.activation(ex[:, :, 0:Hk], lk[:, :, 0:Hk], AF.Exp)
                    mwch = wp.tile([128, CH, WA], f16, tag="mwch")
                    nc.vector.tensor_tensor(
                        out=mwch[:, :, 0:MS].rearrange("p c (h f) -> p c h f", h=Hk),
                        in0=tch[:, :, 0:MS].rearrange("p c (h f) -> p c h f", h=Hk),
                        in1=ex[:, :, 0:Hk].unsqueeze(3).to_broadcast([128, CH, Hk, F]),
                        op=AOP.mult)
                    nc.vector.tensor_scalar_mul(mwch[:, :, MS:MS + Hk], ex[:, :, 0:Hk], exscale)
                    if mid:
                        # edge state: ELU(esum) = min(exp(esum)-1, max(esum, 0))
                        eech = wp.tile([128, CH, F], f16, tag="eech")
                        nc.scalar.activation(eech[:], tch[:, :, 256:320], AF.Exp)
                        nc.vector.tensor_scalar(out=eech[:], in0=eech[:], scalar1=-1.0,
                                                scalar2=None, op0=AOP.add)
                        rrch = wp.tile([128, CH, F], f16, tag="rrch")
                        nc.vector.tensor_scalar(out=rrch[:], in0=tch[:, :, 256:320],
                                                scalar1=0.0, scalar2=None, op0=AOP.max)
                        ench = eech
                        nc.vector.tensor_tensor(out=ench[:], in0=eech[:], in1=rrch[:], op=AOP.min)
                    for tl in range(CH):
                        t = c * CH + tl
                        ps = pseg.tile([SEG, WA], f32, space="PSUM", tag="pseg")
                        nc.tensor.matmul(out=ps[:, 0:WAk],
                                         lhsT=soh[:, t * SEG:(t + 1) * SEG],
                                         rhs=mwch[:, tl, 0:WAk], start=True, stop=True)
                        nc.scalar.copy(out=acst[:, tl, 0:WAk], in_=ps[:, 0:WAk])
                        if mid:
                            pt = ptr.tile([F, 128], f16, space="PSUM", tag="ptr")
                            nc.tensor.transpose(out=pt[:], in_=ench[:, tl, :], identity=ident[:])
                            if tl % 2 == 0:
                                nc.vector.tensor_copy(out=est[:, tl, :], in_=pt[:])
                            else:
                                nc.scalar.copy(out=est[:, tl, :], in_=pt[:])
                    nc.sync.dma_start(
                        out=ACCD[c * CH * SEG:(c + 1) * CH * SEG, 0:WAk].rearrange(
                            "(t s) w -> s t w", s=SEG),
                        in_=acst[:, :, 0:WAk])
                    if mid:
                        nc.sync.dma_start(
                            out=edst[0:F, cs:ce].rearrange("f (t p) -> f t p", p=128),
                            in_=est[:])

            # ------------------------------------------------------------------
            def node_phase(L):
                if L < 3:
                    gn = wp.tile([128, NT_OWN, 320], f32, tag="gn", bufs=1)
                    nc.gpsimd.dma_gather(
                        out_ap=gn[:, 0:8, :], in_ap=ACCD[:, :], idxs_ap=accidx_m[:, 0:64],
                        num_idxs=1024, num_idxs_reg=1024, elem_size=320)
                    nc.gpsimd.dma_gather(
                        out_ap=gn[:, 8:NT_OWN, :], in_ap=ACCD[:, :], idxs_ap=accidx_m[:, 64:88],
                        num_idxs=NPAD - 1024, num_idxs_reg=NPAD - 1024, elem_size=320)
                    xstg = wp.tile([F, NT_OWN, 128], f16, tag="xstg", bufs=1)
                    dstg = wp.tile([128, NT_OWN, 128], f16, tag="dstg", bufs=1)
                    for nt in range(NT_OWN):
                        rec = wp3.tile([128, H], f32, tag="rec")
                        nc.vector.reciprocal(out=rec[:], in_=gn[:, nt, 256:260])
                        pr = wp3.tile([128, 256], f32, tag="pr")
                        nc.vector.tensor_tensor(
                            out=pr[:].rearrange("p (h f) -> p h f", h=H),
                            in0=gn[:, nt, 0:256].rearrange("p (h f) -> p h f", h=H),
                            in1=rec[:].unsqueeze(2).to_broadcast([128, H, F]),
                            op=AOP.mult)
                        xo = wp3.tile([128, F], f32, tag="xo")
                        nc.vector.tensor_reduce(
                            out=xo[:], in_=pr[:].rearrange("p (h f) -> p f h", h=H),
                            axis=mybir.AxisListType.X, op=AOP.add)
                        # ELU(x) = min(exp(x)-1, max(x, 0))
                        xe = wp3.tile([128, F], f16, tag="ee")
                        nc.scalar.activation(xe[:], xo[:], AF.Exp)
                        nc.vector.tensor_scalar(out=xe[:], in0=xe[:], scalar1=-1.0,
                                                scalar2=None, op0=AOP.add)
                        xr = wp3.tile([128, F], f16, tag="rr")
                        nc.vector.tensor_scalar(out=xr[:], in0=xo[:], scalar1=0.0,
                                                scalar2=None, op0=AOP.max)
                        xs = wp3.tile([128, F], f16, tag="en")
                        nc.vector.tensor_tensor(out=xs[:], in0=xe[:], in1=xr[:], op=AOP.min)
                        pt = ptr.tile([F, 128], f16, space="PSUM", tag="ptr")
                        nc.tensor.transpose(out=pt[:], in_=xs[:], identity=ident[:])
                        if nt % 2 == 0:
                            nc.vector.tensor_copy(out=xstg[:, nt, :], in_=pt[:])
                        else:
                            nc.scalar.copy(out=xstg[:, nt, :], in_=pt[:])
                        # next layer's dst-table rows (node-major)
                        pd = pmain.tile([128, WB], f32, space="PSUM", tag="pmain")
                        nc.tensor.matmul(out=pd[:, 0:128],
                                         lhsT=xstg[:, nt, :], rhs=wd[L + 1][:, :],
                                         start=True, stop=True)
                        if nt % 2 == 0:
                            nc.scalar.copy(out=dstg[:, nt, :], in_=pd[:, 0:128])
                        else:
                            nc.vector.tensor_copy(out=dstg[:, nt, :], in_=pd[:, 0:128])
                    nc.sync.dma_start(out=AGIN[:, :].rearrange("f (t p) -> f t p", p=128),
                                      in_=xstg[:])
                    nc.gpsimd.dma_scatter_add(
                        out_ap=DTB[L + 1][:, :], in_ap=dstg[:], idxs_ap=dstscat[:],
                        num_idxs=NPAD, num_idxs_reg=NPAD, elem_size=128)
                    nc.gpsimd.collective_compute(
                        "AllGather", AOP.bypass,
                        replica_groups=[list(range(C))],
                        ins=[AGIN[:]], outs=[CCT[L][:]])
                    build_B(L + 1)
                else:
                    gn = wp.tile([128, NT_OWN, 128], f32, tag="gn", name="gn3", bufs=1)
                    nc.gpsimd.dma_gather(
                        out_ap=gn[:, 0:8, :], in_ap=ACCD[:, 0:128], idxs_ap=accidx_o[:, 0:64],
                        num_idxs=1024, num_idxs_reg=1024, elem_size=128, elem_step=320)
                    nc.gpsimd.dma_gather(
                        out_ap=gn[:, 8:NT_OWN, :], in_ap=ACCD[:, 0:128], idxs_ap=accidx_o[:, 64:88],
                        num_idxs=NPAD - 1024, num_idxs_reg=NPAD - 1024, elem_size=128, elem_step=320)
                    pg = pgp.tile([8, OUT], f32, space="PSUM", tag="pg")
                    for nt in range(NT_OWN):
                        rec = wp3.tile([128, H], f32, tag="rec")
                        nc.vector.reciprocal(out=rec[:, 0:1], in_=gn[:, nt, 64:65])
                        nod = wp3.tile([128, 256], f32, tag="pr", name="nod")
                        nc.vector.tensor_tensor(out=nod[:, 0:OUT], in0=gn[:, nt, 0:OUT],
                                                in1=rec[:, 0:1].to_broadcast([128, OUT]),
                                                op=AOP.mult)
                        nc.tensor.matmul(out=pg[:], lhsT=g1h[:, nt * 8:(nt + 1) * 8],
                                         rhs=nod[:, 0:OUT], start=(nt == 0),
                                         stop=(nt == NT_OWN - 1), skip_group_check=True)
                    og = wp3.tile([8, OUT], f32, tag="og")
                    nc.vector.tensor_copy(out=og[:], in_=pg[:])
                    nc.sync.dma_start(out=out_t.ap(), in_=og[:])

            # ------------------------------------------------------------------
            stage = os.environ.get("KERNEL_STAGE", "full")
            og0 = wp3.tile([8, OUT], f32, tag="og", name="og0")
            nc.vector.memset(og0[:], 0.0)
            nc.sync.dma_start(out=out_t.ap(), in_=og0[:])
            if stage == "full":
                build_dst0()
                build_B(0)
                for L in range(4):
                    edge_phase(L)
                    node_phase(L)
            else:
                n = int(stage)  # 1=B0+dst0, 2=+edge0, 3=+node0, 4=+edge1, ...
                step = 0
                build_dst0()
                build_B(0)
                step += 1
                for L in range(4):
                    if step >= n:
                        break
                    edge_phase(L)
                    step += 1
                    if step >= n:
                        break
                    node_phase(L)
                    step += 1

    nc.compile()
    return nc


def _get_program():
    if "nc" not in _PROGRAM_CACHE:
        _PROGRAM_CACHE["nc"] = _build_program()
    return _PROGRAM_CACHE["nc"]


def kernel(**inputs):
    from concourse.bass_utils import run_bass_kernel_spmd

    nc = _get_program()
    in_maps = _preprocess(inputs)
    trace = bool(int(os.environ.get("KERNEL_TRACE", "0")))
    res = run_bass_kernel_spmd(nc, in_maps, core_ids=list(range(C)), trace=trace)
    _PROGRAM_CACHE["last_result"] = res
    out = np.concatenate([np.asarray(res.results[c]["out"]) for c in range(C)], axis=0)
    return out.astype(np.float32)
